# revision 1
# baseline (speedup 1.0000x reference)
"""DaGCN on 8 Trainium2 NeuronCores (Bass SPMD).

Strategy (graph/data parallel, nodes sharded 8 ways):
  * Each core owns a 6250-node shard (padded to 6272 = 49*128).
  * Feature transforms s = x @ W run as bf16 PE matmuls on host-transposed
    x shards; the resulting per-shard tables are AllGather'ed so every core
    holds the full [50176, 128] bf16 node-feature tables in its DRAM.
  * Edges are assigned to the core owning dst. Per (adjacency, src-half)
    they are sorted by dst block (128 nodes), each block's run padded to
    whole 128-edge chunks. dma_gather (1024 idxs/call) fetches s[src] as
    bf16 messages, edge-on-partition.
  * segment_sum runs on the TensorEngine: per 128-edge chunk a one-hot
    lhsT [128 edges x 128 dst-cols] holding ew (built on DVE from an iota
    compare) is matmul'ed with the message chunk, accumulating each dst
    block in PSUM. No scatter-add (HW races on duplicate indices).
  * Gating/normalization math runs on DVE/ACT over [128, 49, F] shard
    layouts entirely in SBUF.
"""

import math
from contextlib import ExitStack

import ml_dtypes
import numpy as np

import concourse.bacc as bacc
import concourse.bass as bass
import concourse.mybir as mybir
from concourse.bass_utils import run_bass_kernel_spmd

F32 = mybir.dt.float32
BF16 = mybir.dt.bfloat16
I16 = mybir.dt.int16
AOP = mybir.AluOpType
ACT = mybir.ActivationFunctionType

NCORES = 8
N = 50000
NFEAT, NHID, NCLASS = 256, 64, 32
S_CALL = 1024          # idxs per dma_gather call (HW-validated; 2048 hangs)
CALL_CHUNKS = S_CALL // 128
RING = 8               # gather/onehot ring depth (in calls)
NPSUM = 4              # psum block-accumulator ring


def _wrap16(a):
    """[n] int16 -> [128, n//16]: idx i at [i%16, i//16], replicated x8."""
    n = a.shape[0]
    w = a.reshape(n // 16, 16).T.astype(np.int16)
    return np.tile(w, (8, 1)).copy()


def _chunkwrap(a, dtype):
    """[n] -> [128, n//128]: edge i at [i%128, i//128]."""
    n = a.shape[0]
    return np.ascontiguousarray(a.reshape(n // 128, 128).T.astype(dtype))


def _prep_adjacency(src, dst, ew, S, SP, NB, HSPLIT, NROWS):
    """Bucket edges by dst core/block/src-half; returns per-core arrays + CPBs."""
    src = np.asarray(src).astype(np.int64)
    dst = np.asarray(dst).astype(np.int64)
    ew = np.asarray(ew).astype(np.float32)
    core = dst // S
    row = (src // S) * SP + (src % S)       # padded table row
    half = (row >= HSPLIT).astype(np.int64)
    dstrel = dst - core * S
    blk = dstrel // 128
    col = dstrel % 128

    percore = []
    counts = np.zeros((NCORES, 2, NB), np.int64)
    for k in range(NCORES):
        m = core == k
        e = np.lexsort((blk[m], half[m]))   # sort by (half, block)
        r, h, b, c, w = row[m][e], half[m][e], blk[m][e], col[m][e], ew[m][e]
        percore.append((r, h, b, c, w))
        for hh in range(2):
            mm = h == hh
            counts[k, hh] = np.bincount(b[mm], minlength=NB)

    cpb_lo = int(np.ceil(counts[:, 0].max() / 128))
    cpb_hi = int(np.ceil(counts[:, 1].max() / 128))
    cpb_lo = max(cpb_lo, 1)
    cpb_hi = max(cpb_hi, 1)
    ch_lo = -(-NB * cpb_lo // CALL_CHUNKS) * CALL_CHUNKS
    ch_hi = -(-NB * cpb_hi // CALL_CHUNKS) * CALL_CHUNKS
    nslot = (ch_lo + ch_hi) * 128

    out = []
    for k in range(NCORES):
        r, h, b, c, w = percore[k]
        gidx = np.zeros(nslot, np.int64)
        dcol = np.zeros(nslot, np.int64)
        eww = np.zeros(nslot, np.float32)
        for hh, cpb, base_ch, rowbase in ((0, cpb_lo, 0, 0), (1, cpb_hi, ch_lo, HSPLIT)):
            mm = h == hh
            rr, bb, cc, ww = r[mm], b[mm], c[mm], w[mm]
            # position within block run (edges already sorted by block)
            cnt = counts[k, hh]
            offs = np.concatenate(([0], np.cumsum(cnt)))[:-1]
            pos = np.arange(rr.shape[0]) - offs[bb]
            slot = (base_ch + bb * cpb) * 128 + pos
            gidx[slot] = rr - rowbase
            dcol[slot] = cc
            eww[slot] = ww
        out.append((
            _wrap16(gidx),
            _chunkwrap(dcol, np.float32),
            _chunkwrap(eww, np.float32),
        ))
    return out, cpb_lo, cpb_hi, ch_lo, ch_hi, nslot


class Ctr:
    def __init__(self, sem, step=1):
        self.sem, self.n, self.step = sem, 0, step

    def inc(self, inst):
        inst.then_inc(self.sem, self.step)
        self.n += self.step
        return self.n


def _build(S, SP, NB, NROWS, HSPLIT, adjmeta, scalars):
    """adjmeta: {a: (ch_lo, ch_hi, nslot)}; scalars: g1b,g2b,h1b,h2b floats."""
    nc = bacc.Bacc("TRN2", num_devices=NCORES, num_swdge_queues=2)
    g1b, g2b, h1b, h2b = scalars
    ncal_max = max((m[0] + m[1]) // CALL_CHUNKS for m in adjmeta.values())
    nslot_max = max(m[2] for m in adjmeta.values())
    nch_max = nslot_max // 128

    # ---------------- I/O ----------------
    din = {}
    for v in ("xt1a", "xt1b", "xt2a", "xt2b"):
        din[v] = nc.dram_tensor(v, [128, 2, SP], BF16, kind="ExternalInput")
    din["w1a"] = nc.dram_tensor("w1a", [128, 2, NHID], BF16, kind="ExternalInput")
    din["w1b"] = nc.dram_tensor("w1b", [128, 2, NHID], BF16, kind="ExternalInput")
    din["w2"] = nc.dram_tensor("w2", [128, 64], BF16, kind="ExternalInput")
    din["iota"] = nc.dram_tensor("iota", [128, 128], BF16, kind="ExternalInput")
    din["idf"] = nc.dram_tensor("idf", [128, 128], F32, kind="ExternalInput")
    din["idb"] = nc.dram_tensor("idb", [128, 128], BF16, kind="ExternalInput")
    din["g1w"] = nc.dram_tensor("g1w", [128, 128], F32, kind="ExternalInput")
    din["g2w"] = nc.dram_tensor("g2w", [128, 128], F32, kind="ExternalInput")
    din["h1w"] = nc.dram_tensor("h1w", [128, 64], F32, kind="ExternalInput")
    din["h2w"] = nc.dram_tensor("h2w", [128, 64], F32, kind="ExternalInput")
    din["b1r"] = nc.dram_tensor("b1r", [128, 128], F32, kind="ExternalInput")
    din["b2r"] = nc.dram_tensor("b2r", [128, 64], F32, kind="ExternalInput")
    for a in (1, 2):
        ns = adjmeta[a][2]
        din[f"gidx{a}"] = nc.dram_tensor(f"gidx{a}", [128, ns // 16], I16, kind="ExternalInput")
        din[f"dst{a}"] = nc.dram_tensor(f"dst{a}", [128, ns // 128], F32, kind="ExternalInput")
        din[f"eww{a}"] = nc.dram_tensor(f"eww{a}", [128, ns // 128], F32, kind="ExternalInput")
    out_o = nc.dram_tensor("out_o", [SP, NCLASS], F32, kind="ExternalOutput")
    p1_o = nc.dram_tensor("p1_o", [SP, NCLASS], F32, kind="ExternalOutput")
    p2_o = nc.dram_tensor("p2_o", [SP, NCLASS], F32, kind="ExternalOutput")

    t_in = {t: nc.dram_tensor(f"t{t}in", [SP, 128], BF16) for t in (1, 2, 3)}
    t_full = {t: nc.dram_tensor(f"t{t}full", [NROWS, 128], BF16, addr_space="Shared")
              for t in (1, 2, 3)}

    ctx = ExitStack()
    sb = lambda name, shape, dt: ctx.enter_context(nc.sbuf_tensor(name, shape, dt))
    ps = lambda name, shape: ctx.enter_context(nc.psum_tensor(name, shape, F32))
    sem = lambda name: ctx.enter_context(nc.semaphore(name))

    # ---------------- SBUF ----------------
    c_w1a = sb("c_w1a", [128, 2, NHID], BF16)
    c_w1b = sb("c_w1b", [128, 2, NHID], BF16)
    c_w2 = sb("c_w2", [128, 64], BF16)
    c_iota = sb("c_iota", [128, 128], BF16)
    c_idf = sb("c_idf", [128, 128], F32)
    c_idb = sb("c_idb", [128, 128], BF16)
    c_g1w = sb("c_g1w", [128, 128], F32)
    c_g2w = sb("c_g2w", [128, 128], F32)
    c_h1w = sb("c_h1w", [128, 64], F32)
    c_h2w = sb("c_h2w", [128, 64], F32)
    c_b1r = sb("c_b1r", [128, 128], F32)
    c_b2r = sb("c_b2r", [128, 64], F32)

    sT = sb("sT", [128, SP], BF16)
    tstage = sb("tstage", [128, NB, 128], BF16)
    agg1 = sb("agg1", [128, NB, 128], F32)
    agg2 = sb("agg2", [128, NB, 128], F32)
    tmp = sb("tmp", [128, NB, 128], F32)
    xtt = sb("xtt", [128, 128], BF16)
    lamv = {nm: sb(nm, [128, NB], F32)
            for nm in ("l1", "l2", "lsum", "w0", "w1")}
    cbias = sb("cbias", [128, 4], F32)
    sbA = ExitStack()
    xta = sbA.enter_context(nc.sbuf_tensor("xta", [128, 2, SP], BF16))
    xtb = sbA.enter_context(nc.sbuf_tensor("xtb", [128, 2, SP], BF16))

    psA = ExitStack()
    mm_ps = [psA.enter_context(nc.psum_tensor(f"mm_ps{i}", [128, 512], F32))
             for i in range(2)]
    trb_ps = [psA.enter_context(nc.psum_tensor(f"trb_ps{i}", [128, 128], BF16))
              for i in range(2)]

    io = Ctr(sem("io"), 16)        # sync-engine DMAs
    gsems = [Ctr(sem(f"g{i}"), 16) for i in range(RING)]  # per-ring-slot gathers
    ccs = [Ctr(sem(f"cc{i}"), 1) for i in range(3)]   # one sem per collective
    pe = Ctr(sem("pe"), 1)         # PE milestones
    dv = Ctr(sem("dv"), 1)         # DVE milestones
    ac = Ctr(sem("ac"), 1)         # ACT milestones

    SY, PE, DV, AC, GP = nc.sync, nc.tensor, nc.vector, nc.scalar, nc.gpsimd

    def fence():
        # sync engine waits for all its issued DMAs: later cross-engine
        # io-threshold waits become unambiguous (no completion reordering).
        SY.wait_ge(io.sem, io.n)

    # =========== Phase A: constants + s tables ===========
    for bi, bval in enumerate((g1b, g2b, h1b, h2b)):
        nc.vector.memset(cbias[:, bi:bi + 1], float(bval))
    for name, t in (("w1a", c_w1a), ("w1b", c_w1b), ("w2", c_w2), ("iota", c_iota),
                    ("idf", c_idf), ("idb", c_idb), ("g1w", c_g1w), ("g2w", c_g2w),
                    ("h1w", c_h1w), ("h2w", c_h2w), ("b1r", c_b1r), ("b2r", c_b2r)):
        io.inc(SY.dma_start(t[:], din[name][:]))
    consts_io = io.n

    nsl = [(j * 512, min(512, SP - j * 512)) for j in range((SP + 511) // 512)]

    def s_table(tbl, va, vb, wa, wb, pe_wait_extra):
        """matmul s = [x_va@W1a | x_vb@W1b] -> transpose -> tstage -> DMA t_in."""
        io.inc(SY.dma_start(xta[:], din[va][:]))
        io.inc(SY.dma_start(xtb[:], din[vb][:]))
        xload = io.n
        copies = []
        for j, (o, n) in enumerate(nsl):
            p = mm_ps[j % 2]
            if j == 0:
                PE.wait_ge(io.sem, xload)
                if pe_wait_extra is not None:
                    PE.wait_ge(dv.sem, pe_wait_extra)
            if j >= 2 and copies[j - 2] is not None:
                PE.wait_ge(dv.sem, copies[j - 2])
            for xt, w, prow in ((xta, wa, 0), (xtb, wb, 64)):
                for cch in range(2):
                    last = PE.matmul(p[prow:prow + 64, 0:n], w[:, cch, :],
                                     xt[:, cch, o:o + n],
                                     start=(cch == 0), stop=(cch == 1))
            pe.inc(last)
            pev = pe.n
            DV.wait_ge(pe.sem, pev)
            cp = DV.tensor_copy(sT[:, o:o + n], p[:, 0:n])
            dv.inc(cp)
            copies.append(dv.n)
        # transposes into tstage
        trc = {}
        for t in range(NB):
            p = trb_ps[t % 2]
            PE.wait_ge(dv.sem, copies[-1])
            if t >= 2:
                PE.wait_ge(dv.sem, trc[t - 2])
            pe.inc(PE.transpose(p[:], sT[:, t * 128:(t + 1) * 128], c_idb[:]))
            DV.wait_ge(pe.sem, pe.n)
            dv.inc(DV.tensor_copy(tstage[:, t, :], p[:]))
            trc[t] = dv.n
        SY.wait_ge(dv.sem, dv.n)
        io.inc(SY.dma_start(
            t_in[tbl][:].rearrange("(t p) f -> p t f", p=128), tstage[:]))
        fence()
        return io.n, pe.n

    t1_io, t1_pe = s_table(1, "xt1a", "xt1b", c_w1a, c_w1b, None)
    # table2 reuses xta/xtb: its x DMAs must wait for table1's matmuls;
    # emit the waits on the sync engine before the loads.
    SY.wait_ge(pe.sem, t1_pe)
    # tstage reuse: table2's transpose copies (DVE) wait t1in DMA done
    DV.wait_ge(io.sem, t1_io)
    t2_io, t2_pe = s_table(2, "xt2a", "xt2b", c_w1a, c_w1b, None)

    pe_phaseA = pe.n
    GP.wait_ge(io.sem, t1_io)
    ccs[0].inc(GP.collective_compute(
        "AllGather", AOP.bypass, replica_groups=[list(range(NCORES))],
        ins=[t_in[1][:]], outs=[t_full[1][:]]))
    GP.wait_ge(io.sem, t2_io)
    ccs[1].inc(GP.collective_compute(
        "AllGather", AOP.bypass, replica_groups=[list(range(NCORES))],
        ins=[t_in[2][:]], outs=[t_full[2][:]]))

    # =========== edge pass machinery ===========
    psA.close()  # phase-A PSUM freed; per-engine program order makes reuse safe
    sbA.close()  # xta/xtb freed -> reused by edge buffers (guarded by waits below)
    blk_ps = [ps(f"blk_ps{i}", [128, 128]) for i in range(NPSUM)]
    tr_ps = [ps(f"tr_ps{i}", [128, 128]) for i in range(2)]
    prop1 = sb("prop1", [128, NB, 64], F32)
    prop2 = sb("prop2", [128, NB, 64], F32)
    msg = sb("msg", [128, RING * CALL_CHUNKS, 128], BF16)
    ohr = sb("ohr", [128, RING * CALL_CHUNKS, 128], BF16)
    gidx_sb = sb("gidx_sb", [128, nslot_max // 16], I16)
    dst_sb = sb("dst_sb", [128, nch_max], F32)
    ew_sb = sb("ew_sb", [128, nch_max], F32)
    gcall = [0]      # global gather call counter
    pe_cons_vals = []
    npass = [0]
    psum_last = [None] * NPSUM  # (sem, val) of last copy freeing each psum slot

    def edge_pass(adj, table, F_rhs, dest, cc_need, ch_lo, ch_hi, cpb_lo, cpb_hi,
                  add_mode):
        """One (layer, adjacency) pass: lo half then hi half."""
        ns = (ch_lo + ch_hi) * 128
        # WAR: don't overwrite idx arrays while a previous pass still reads
        # them, nor the freed xta/xtb space while phase-A PE still reads it
        for gs in gsems:
            SY.wait_ge(gs.sem, gs.n)
        SY.wait_ge(dv.sem, dv.n)
        if npass[0] == 0:
            SY.wait_ge(pe.sem, pe_phaseA)
        io.inc(SY.dma_start(gidx_sb[:, 0:ns // 16], din[f"gidx{adj}"][:]))
        io.inc(SY.dma_start(dst_sb[:, 0:ns // 128], din[f"dst{adj}"][:]))
        io.inc(SY.dma_start(ew_sb[:, 0:ns // 128], din[f"eww{adj}"][:]))
        fence()
        idx_io = io.n

        lo_copy_ac = {}
        GP.wait_ge(ccs[cc_need].sem, 1)
        if npass[0] == 0:
            GP.wait_ge(pe.sem, pe_phaseA)   # msg ring aliases freed xta/xtb
            DV.wait_ge(pe.sem, pe_phaseA)   # ohr ring likewise
        npass[0] += 1
        for half, ch, cpb, base in ((0, ch_lo, cpb_lo, 0), (1, ch_hi, cpb_hi, HSPLIT)):
            ch0 = 0 if half == 0 else ch_lo  # chunk offset in the arrays
            tab = table[base:NROWS] if half == 1 else table[0:HSPLIT]
            blk_of = lambda c: min(c // cpb, NB - 1)
            endc = lambda b: (b + 1) * cpb - 1 if b < NB - 1 else ch - 1
            for j in range(ch // CALL_CHUNKS):
                rj = (gcall[0] % RING) * CALL_CHUNKS
                GP.wait_ge(io.sem, idx_io)
                if len(pe_cons_vals) >= RING:
                    GP.wait_ge(pe.sem, pe_cons_vals[-RING])
                gslot = gcall[0] % RING
                g = GP.dma_gather(
                    msg[:, rj:rj + CALL_CHUNKS, :], tab,
                    gidx_sb[:, (ch0 * 8 + j * S_CALL // 16):(ch0 * 8 + (j + 1) * S_CALL // 16)],
                    S_CALL, S_CALL, 128, queue_num=gcall[0] % 2)
                gsems[gslot].inc(g)
                gv = gsems[gslot].n
                # onehot build
                DV.wait_ge(io.sem, idx_io)
                if len(pe_cons_vals) >= RING:
                    DV.wait_ge(pe.sem, pe_cons_vals[-RING])
                cbase = ch0 + j * CALL_CHUNKS
                for c8 in range(CALL_CHUNKS):
                    ts = DV.tensor_scalar(
                        ohr[:, rj + c8, :], c_iota[:],
                        dst_sb[:, cbase + c8:cbase + c8 + 1],
                        ew_sb[:, cbase + c8:cbase + c8 + 1],
                        op0=AOP.is_equal, op1=AOP.mult)
                dv.inc(ts)
                ohv = dv.n
                # matmuls
                PE.wait_ge(gsems[gslot].sem, gv)
                PE.wait_ge(dv.sem, ohv)
                last_was_end = False
                for c8 in range(CALL_CHUNKS):
                    c = j * CALL_CHUNKS + c8
                    b = blk_of(c)
                    slot = b % NPSUM
                    p = blk_ps[slot]
                    st = (c == b * cpb)
                    if st and psum_last[slot] is not None:
                        eng, val = psum_last[slot]
                        PE.wait_ge(dv.sem if eng == "dv" else ac.sem, val)
                    mmi = PE.matmul(p[:, 0:F_rhs],
                                    ohr[:, rj + c8, :],
                                    msg[:, rj + c8, 0:F_rhs],
                                    start=st, stop=(c == endc(b)))
                    last_was_end = (c == endc(b))
                    if last_was_end:
                        pe.inc(mmi)
                        if add_mode or half == 1:
                            DV.wait_ge(pe.sem, pe.n)
                            DV.wait_ge(ac.sem, lo_copy_ac[b])
                            cpi = DV.tensor_tensor(dest[:, b, 0:F_rhs],
                                                   dest[:, b, 0:F_rhs],
                                                   p[:, 0:F_rhs], op=AOP.add)
                            dv.inc(cpi)
                            psum_last[slot] = ("dv", dv.n)
                        else:
                            AC.wait_ge(pe.sem, pe.n)
                            cpi = AC.activation(dest[:, b, 0:F_rhs],
                                                p[:, 0:F_rhs], ACT.Copy)
                            ac.inc(cpi)
                            psum_last[slot] = ("ac", ac.n)
                            lo_copy_ac[b] = ac.n
                if not last_was_end:
                    pe.inc(mmi)
                pe_cons_vals.append(pe.n)
                gcall[0] += 1
        return dv.n

    m1 = adjmeta[1]
    m2 = adjmeta[2]
    edge_pass(1, t_full[1], 128, agg1, 0, m1[0], m1[1], m1[3], m1[4], False)
    edge_pass(2, t_full[2], 128, agg2, 1, m2[0], m2[1], m2[3], m2[4], False)

    # =========== Phase C: mid gating + L2 table ===========
    DV.drain()
    b1b = c_b1r[:, None, :].broadcast_to([128, NB, 128])
    DV.tensor_tensor(agg1[:], agg1[:], b1b, op=AOP.add)
    DV.tensor_tensor(agg2[:], agg2[:], b1b, op=AOP.add)
    DV.drain()
    DV.tensor_scalar(agg1[:], agg1[:], 0.0, None, op0=AOP.max)
    DV.tensor_scalar(agg2[:], agg2[:], 0.0, None, op0=AOP.max)
    g1b_b = c_g1w[:, None, :].broadcast_to([128, NB, 128])
    g2b_b = c_g2w[:, None, :].broadcast_to([128, NB, 128])
    DV.drain()
    DV.tensor_tensor(tmp[:], agg1[:], g1b_b, op=AOP.mult)
    DV.drain()
    DV.tensor_reduce(lamv["l1"][:], tmp[:], axis=mybir.AxisListType.X, op=AOP.add)
    DV.drain()
    DV.tensor_tensor(tmp[:], agg2[:], g2b_b, op=AOP.mult)
    DV.drain()
    dv.inc(DV.tensor_reduce(lamv["l2"][:], tmp[:], axis=mybir.AxisListType.X,
                            op=AOP.add))
    AC.wait_ge(dv.sem, dv.n)
    AC.activation(lamv["l1"][:], lamv["l1"][:], ACT.Sigmoid, bias=cbias[:, 0:1])
    ac.inc(AC.activation(lamv["l2"][:], lamv["l2"][:], ACT.Sigmoid, bias=cbias[:, 1:2]))
    DV.wait_ge(ac.sem, ac.n)
    DV.tensor_tensor(lamv["lsum"][:], lamv["l1"][:], lamv["l2"][:], op=AOP.add)
    DV.drain()
    DV.tensor_scalar(lamv["lsum"][:], lamv["lsum"][:], 1e-12, None, op0=AOP.max)
    DV.drain()
    DV.reciprocal(lamv["lsum"][:], lamv["lsum"][:])
    DV.drain()
    DV.tensor_tensor(lamv["w0"][:], lamv["l1"][:], lamv["lsum"][:], op=AOP.mult)
    DV.tensor_tensor(lamv["w1"][:], lamv["l2"][:], lamv["lsum"][:], op=AOP.mult)
    w0b = lamv["w0"][:, :, None].broadcast_to([128, NB, 128])
    w1b_ = lamv["w1"][:, :, None].broadcast_to([128, NB, 128])
    DV.drain()
    DV.tensor_tensor(agg1[:], agg1[:], w0b, op=AOP.mult)
    DV.tensor_tensor(agg2[:], agg2[:], w1b_, op=AOP.mult)
    DV.drain()
    DV.tensor_tensor(agg1[:], agg1[:], agg2[:], op=AOP.add)   # x -> agg1
    DV.drain()
    dv.inc(DV.memset(tstage[:], 0))
    xfin = dv.n

    # L2 table: s2 = x @ W2 (pad to 64 cols), rows bf16-padded to 128
    s2_ps = tr_ps  # reuse [128,128] psum tiles
    stc = {}
    for t in range(NB):
        p = s2_ps[t % 2]
        if t == 0:
            PE.wait_ge(dv.sem, xfin)
        if t >= 2:
            PE.wait_ge(dv.sem, stc[t - 2])
        pe.inc(PE.transpose(p[:], agg1[:, t, :], c_idf[:]))
        DV.wait_ge(pe.sem, pe.n)
        dv.inc(DV.tensor_copy(xtt[:], p[:]))
        PE.wait_ge(dv.sem, dv.n)
        pe.inc(PE.matmul(p[:, 0:64], xtt[:], c_w2[:], start=True, stop=True))
        DV.wait_ge(pe.sem, pe.n)
        dv.inc(DV.tensor_copy(tstage[:, t, 0:64], p[:, 0:64]))
        stc[t] = dv.n
    SY.wait_ge(dv.sem, dv.n)
    io.inc(SY.dma_start(t_in[3][:].rearrange("(t p) f -> p t f", p=128), tstage[:]))
    fence()
    GP.wait_ge(io.sem, io.n)
    ccs[2].inc(GP.collective_compute(
        "AllGather", AOP.bypass, replica_groups=[list(range(NCORES))],
        ins=[t_in[3][:]], outs=[t_full[3][:]]))

    # =========== L2 edge passes ===========
    edge_pass(1, t_full[3], 64, prop1, 2, m1[0], m1[1], m1[3], m1[4], False)
    edge_pass(2, t_full[3], 64, prop2, 2, m2[0], m2[1], m2[3], m2[4], False)

    # =========== Phase F: final gating + outputs ===========
    DV.drain()
    b2b = c_b2r[:, None, :].broadcast_to([128, NB, 64])
    DV.tensor_tensor(prop1[:], prop1[:], b2b, op=AOP.add)
    DV.drain()
    dv.inc(DV.tensor_tensor(prop2[:], prop2[:], b2b, op=AOP.add))
    pfin = dv.n
    SY.wait_ge(dv.sem, pfin)
    io.inc(SY.dma_start(p1_o[:].rearrange("(t p) f -> p t f", p=128),
                        prop1[:, :, 0:NCLASS]))
    io.inc(SY.dma_start(p2_o[:].rearrange("(t p) f -> p t f", p=128),
                        prop2[:, :, 0:NCLASS]))
    fence()
    pout_io = io.n
    h1b_b = c_h1w[:, None, :].broadcast_to([128, NB, 64])
    h2b_b = c_h2w[:, None, :].broadcast_to([128, NB, 64])
    t64 = tmp[:, :, 0:64]
    DV.tensor_tensor(t64, prop1[:], h1b_b, op=AOP.mult)
    DV.drain()
    DV.tensor_reduce(lamv["l1"][:], t64, axis=mybir.AxisListType.X, op=AOP.add)
    DV.drain()
    DV.tensor_tensor(t64, prop2[:], h2b_b, op=AOP.mult)
    DV.drain()
    dv.inc(DV.tensor_reduce(lamv["l2"][:], t64, axis=mybir.AxisListType.X,
                            op=AOP.add))
    AC.wait_ge(dv.sem, dv.n)
    AC.activation(lamv["l1"][:], lamv["l1"][:], ACT.Sigmoid, bias=cbias[:, 2:3])
    ac.inc(AC.activation(lamv["l2"][:], lamv["l2"][:], ACT.Sigmoid, bias=cbias[:, 3:4]))
    DV.wait_ge(ac.sem, ac.n)
    DV.tensor_tensor(lamv["lsum"][:], lamv["l1"][:], lamv["l2"][:], op=AOP.add)
    DV.drain()
    DV.tensor_scalar(lamv["lsum"][:], lamv["lsum"][:], 1e-12, None, op0=AOP.max)
    DV.drain()
    DV.reciprocal(lamv["lsum"][:], lamv["lsum"][:])
    DV.drain()
    DV.tensor_tensor(lamv["w0"][:], lamv["l1"][:], lamv["lsum"][:], op=AOP.mult)
    DV.tensor_tensor(lamv["w1"][:], lamv["l2"][:], lamv["lsum"][:], op=AOP.mult)
    DV.drain()
    w0b6 = lamv["w0"][:, :, None].broadcast_to([128, NB, 64])
    w1b6 = lamv["w1"][:, :, None].broadcast_to([128, NB, 64])
    DV.wait_ge(io.sem, pout_io)  # don't clobber props mid-DMA
    DV.tensor_tensor(t64, prop1[:], w0b6, op=AOP.mult)
    DV.tensor_tensor(prop2[:], prop2[:], w1b6, op=AOP.mult)
    DV.drain()
    dv.inc(DV.tensor_tensor(t64, t64, prop2[:], op=AOP.add))
    SY.wait_ge(dv.sem, dv.n)
    io.inc(SY.dma_start(out_o[:].rearrange("(t p) f -> p t f", p=128),
                        tmp[:, :, 0:NCLASS]))
    SY.wait_ge(io.sem, io.n)

    nc.compile()
    ctx.close()
    return nc


def _run(inputs, sim=False):
    S = inputs["x1a"].shape[0] // NCORES
    NB = -(-S // 128)
    SP = NB * 128
    NROWS = NCORES * SP
    HSPLIT = min(32768, NROWS // 2 // 128 * 128)

    adj = {}
    adjmeta = {}
    for a in (1, 2):
        out, cpb_lo, cpb_hi, ch_lo, ch_hi, nslot = _prep_adjacency(
            inputs[f"src{a}"], inputs[f"dst{a}"], inputs[f"ew{a}"],
            S, SP, NB, HSPLIT, NROWS)
        adj[a] = out
        adjmeta[a] = (ch_lo, ch_hi, nslot, cpb_lo, cpb_hi)

    scalars = (float(np.asarray(inputs["g1b"]).ravel()[0]),
               float(np.asarray(inputs["g2b"]).ravel()[0]),
               float(np.asarray(inputs["h1b"]).ravel()[0]),
               float(np.asarray(inputs["h2b"]).ravel()[0]))
    nc = _build(S, SP, NB, NROWS, HSPLIT, adjmeta, scalars)

    bf = ml_dtypes.bfloat16
    f32 = np.float32

    def wfmt(w):  # [256, 64] -> [128, 2, 64] bf16
        return np.ascontiguousarray(
            np.asarray(w, f32).reshape(2, 128, NHID).transpose(1, 0, 2)).astype(bf)

    w2pad = np.zeros((128, 64), f32)
    w2pad[:, :NCLASS] = np.asarray(inputs["W2"], f32)
    iota = np.tile(np.arange(128, dtype=f32), (128, 1))
    ident = np.eye(128, dtype=f32)
    g1w = np.tile(np.asarray(inputs["g1w"], f32).ravel(), (128, 1))
    g2w = np.tile(np.asarray(inputs["g2w"], f32).ravel(), (128, 1))
    h1w = np.zeros((128, 64), f32)
    h1w[:, :NCLASS] = np.asarray(inputs["h1w"], f32).ravel()
    h2w = np.zeros((128, 64), f32)
    h2w[:, :NCLASS] = np.asarray(inputs["h2w"], f32).ravel()
    b1r = np.tile(np.concatenate([np.asarray(inputs["b1a"], f32).ravel(),
                                  np.asarray(inputs["b1b"], f32).ravel()]), (128, 1))
    b2r = np.zeros((128, 64), f32)
    b2r[:, :NCLASS] = np.asarray(inputs["b2"], f32).ravel()

    common = dict(
        w1a=wfmt(inputs["W1a"]), w1b=wfmt(inputs["W1b"]),
        w2=w2pad.astype(bf), iota=iota.astype(bf), idf=ident,
        idb=ident.astype(bf), g1w=g1w, g2w=g2w, h1w=h1w, h2w=h2w,
        b1r=b1r, b2r=b2r)

    def xfmt(x, k):  # shard k, pad, transpose -> [128, 2, SP] bf16
        xs = np.asarray(x, f32)[k * S:(k + 1) * S]
        xp = np.zeros((SP, NFEAT), f32)
        xp[:S] = xs
        xt = xp.T.reshape(2, 128, SP).transpose(1, 0, 2)
        return np.ascontiguousarray(xt).astype(bf)

    in_maps = []
    for k in range(NCORES):
        m = dict(common)
        for v, key in (("xt1a", "x1a"), ("xt1b", "x1b"),
                       ("xt2a", "x2a"), ("xt2b", "x2b")):
            m[v] = xfmt(inputs[key], k)
        for a in (1, 2):
            g, d, e = adj[a][k]
            m[f"gidx{a}"] = g
            m[f"dst{a}"] = d
            m[f"eww{a}"] = e
        in_maps.append(m)

    global LAST_EXEC_NS
    if sim:
        from concourse.bass_interp import MultiCoreSim
        msim = MultiCoreSim(nc, NCORES)
        for k in range(NCORES):
            for name, arr in in_maps[k].items():
                msim.cores[k].tensor(name)[:] = arr
        msim.simulate()
        results = [{nm: msim.cores[k].tensor(nm).copy()
                    for nm in ("out_o", "p1_o", "p2_o")} for k in range(NCORES)]
    else:
        import os
        import time as _time
        trace = bool(os.environ.get("KERNEL_TRACE"))
        r = run_bass_kernel_spmd(nc, in_maps, list(range(NCORES)), trace=trace)
        LAST_EXEC_NS = r.exec_time_ns
        results = r.results
        if os.environ.get("KERNEL_REPEAT"):
            t0 = _time.perf_counter()
            run_bass_kernel_spmd(nc, in_maps, list(range(NCORES)))
            global LAST_WALL2_S
            LAST_WALL2_S = _time.perf_counter() - t0

    outs = []
    for nm in ("out_o", "p1_o", "p2_o"):
        outs.append(np.concatenate([results[k][nm][:S] for k in range(NCORES)],
                    axis=0).astype(np.float32))
    return tuple(outs)


LAST_EXEC_NS = None
LAST_WALL2_S = None


def kernel(**inputs):
    return _run(inputs, sim=False)



# revision 15
# speedup vs baseline: 1.6672x; 1.6672x over previous
"""DaGCN on 8 Trainium2 NeuronCores (Bass SPMD), src-sharded + ReduceScatter.

Strategy (v2):
  * Edges are assigned to the core owning SRC. Each core computes its local
    s-tables (s = x @ W for its 6272-row node shard) -- no AllGather needed:
    every gather is from the core's own DRAM table with int16 local rows.
  * segment_sum over GLOBAL dst: per 128-edge chunk a one-hot lhsT built on
    DVE is matmul'ed with the gathered messages, accumulating each of the 392
    global dst blocks in PSUM (groups of 7 blocks per 2-bank PSUM tile).
  * Finished PSUM groups are copied to bf16 staging (ACT engine) and DMA'd
    into a [8, 2, SP, F] partial buffer laid out rank-major, so ONE merged
    ReduceScatter per layer (both adjacencies) delivers each dst-owner its
    [2, SP, F] reduced shard. Collective output is 8x smaller than the
    baseline AllGather, and only 2 collectives total (vs 3 big AllGathers).
  * Gating/normalization math runs on DVE/GPSIMD/ACT over [128, 49, F]
    shard layouts; the L2 table s2 = x @ W2 is built locally (transpose
    pipeline), again no collective.
"""

from contextlib import ExitStack

import ml_dtypes
import numpy as np

import concourse.bacc as bacc
import concourse.mybir as mybir
from concourse.bass_utils import run_bass_kernel_spmd

F32 = mybir.dt.float32
BF16 = mybir.dt.bfloat16
I16 = mybir.dt.int16
AOP = mybir.AluOpType
ACTF = mybir.ActivationFunctionType

NCORES = 8
N = 50000
NFEAT, NHID, NCLASS = 256, 64, 32
S = N // NCORES                  # 6250 nodes per core
NB = -(-S // 128)                # 49 blocks per core
SP = NB * 128                    # 6272 padded rows
NBF = NCORES * NB                # 392 global dst blocks
GROUP = 7                        # blocks per PSUM group (49 = 7*7)
NPG = 4                          # PSUM tile ring ([128,7,128] f32 = 2 banks)
S_CALL = 1024                    # idxs per dma_gather call
CALL_CHUNKS = S_CALL // 128
RING = 8                         # gather ring depth (in calls)


def _wrap16(a):
    """[n] int -> [128, n//16]: idx i at [i%16, i//16], replicated x8."""
    n = a.shape[0]
    w = a.reshape(n // 16, 16).T.astype(np.int16)
    return np.tile(w, (8, 1)).copy()


def _chunkwrap(a, dtype):
    """[n] -> [128, n//128]: edge i at [i%128, i//128]."""
    n = a.shape[0]
    return np.ascontiguousarray(a.reshape(n // 128, 128).T.astype(dtype))


def _prep_adjacency(src, dst, ew):
    """Bucket edges by (src core, global dst block); shared chunk schedule.

    Returns (per-core arrays, cpb[NBF] chunk counts per block).
    """
    src = np.asarray(src).astype(np.int64)
    dst = np.asarray(dst).astype(np.int64)
    ew = np.asarray(ew).astype(np.float32)
    core = src // S
    lrow = src - core * S                    # local gather row (0..S-1)
    dc = dst // S
    dloc = dst - dc * S
    blk = dc * NB + dloc // 128              # global dst block 0..NBF-1
    col = dloc % 128

    percore = []
    counts = np.zeros((NCORES, NBF), np.int64)
    for k in range(NCORES):
        m = core == k
        e = np.argsort(blk[m], kind='stable')
        percore.append((lrow[m][e], blk[m][e], col[m][e], ew[m][e]))
        counts[k] = np.bincount(blk[m], minlength=NBF)

    cpb = np.maximum(np.ceil(counts / 128).astype(np.int64).max(axis=0), 1)
    pad = (-int(cpb.sum())) % CALL_CHUNKS
    cpb[-1] += pad
    nch = int(cpb.sum())
    choff = np.concatenate(([0], np.cumsum(cpb)))[:-1]   # chunk offset per blk

    out = []
    for k in range(NCORES):
        r, b, c, w = percore[k]
        gidx = np.zeros(nch * 128, np.int64)
        dcol = np.zeros(nch * 128, np.int64)
        eww = np.zeros(nch * 128, np.float32)
        cnt = counts[k]
        offs = np.concatenate(([0], np.cumsum(cnt)))[:-1]
        pos = np.arange(r.shape[0]) - offs[b]
        slot = choff[b] * 128 + pos
        gidx[slot] = r
        dcol[slot] = c
        eww[slot] = w
        out.append((
            _wrap16(gidx),
            _chunkwrap(dcol, np.float32),
            _chunkwrap(eww, np.float32),
        ))
    return out, cpb


class Ctr:
    def __init__(self, sem, step=1):
        self.sem, self.n, self.step = sem, 0, step

    def inc(self, inst):
        inst.then_inc(self.sem, self.step)
        self.n += self.step
        return self.n


def _build(cpbs, scalars):
    """cpbs: {a: cpb array [NBF]}; scalars: g1b,g2b,h1b,h2b floats."""
    nc = bacc.Bacc("TRN2", num_devices=NCORES, num_swdge_queues=2)
    g1b, g2b, h1b, h2b = scalars
    nch = {a: int(cpbs[a].sum()) for a in (1, 2)}

    # ---------------- I/O ----------------
    din = {}
    for v in ("xt1a", "xt1b", "xt2a", "xt2b"):
        din[v] = nc.dram_tensor(v, [128, 2, SP], BF16, kind="ExternalInput")
    din["w1a"] = nc.dram_tensor("w1a", [128, 2, NHID], BF16, kind="ExternalInput")
    din["w1b"] = nc.dram_tensor("w1b", [128, 2, NHID], BF16, kind="ExternalInput")
    din["w2"] = nc.dram_tensor("w2", [128, 64], BF16, kind="ExternalInput")
    din["iota"] = nc.dram_tensor("iota", [128, 128], BF16, kind="ExternalInput")
    din["idb"] = nc.dram_tensor("idb", [128, 128], BF16, kind="ExternalInput")
    din["g1w"] = nc.dram_tensor("g1w", [128, 128], BF16, kind="ExternalInput")
    din["g2w"] = nc.dram_tensor("g2w", [128, 128], BF16, kind="ExternalInput")
    din["h1w"] = nc.dram_tensor("h1w", [128, 64], F32, kind="ExternalInput")
    din["h2w"] = nc.dram_tensor("h2w", [128, 64], F32, kind="ExternalInput")
    din["b1r"] = nc.dram_tensor("b1r", [128, 128], BF16, kind="ExternalInput")
    din["b2r"] = nc.dram_tensor("b2r", [128, 64], F32, kind="ExternalInput")
    for a in (1, 2):
        ns = nch[a] * 128
        din[f"gidx{a}"] = nc.dram_tensor(f"gidx{a}", [128, ns // 16], I16,
                                         kind="ExternalInput")
        din[f"dst{a}"] = nc.dram_tensor(f"dst{a}", [128, ns // 128], F32,
                                        kind="ExternalInput")
        din[f"eww{a}"] = nc.dram_tensor(f"eww{a}", [128, ns // 128], F32,
                                        kind="ExternalInput")
    out_o = nc.dram_tensor("out_o", [SP, NCLASS], F32, kind="ExternalOutput")
    p1_o = nc.dram_tensor("p1_o", [SP, NCLASS], F32, kind="ExternalOutput")
    p2_o = nc.dram_tensor("p2_o", [SP, NCLASS], F32, kind="ExternalOutput")

    tdr = {t: nc.dram_tensor(f"t{t}", [SP, 128], BF16) for t in (1, 2, 3)}
    part1 = nc.dram_tensor("part1", [NCORES * 2 * SP, 128], BF16)
    part2 = nc.dram_tensor("part2", [NCORES * 2 * SP, 64], BF16)
    o12 = nc.dram_tensor("o12", [2 * SP, 128], BF16)
    o34 = nc.dram_tensor("o34", [2 * SP, 64], BF16)

    ctx = ExitStack()
    sb = lambda name, shape, dt: ctx.enter_context(nc.sbuf_tensor(name, shape, dt))
    sem = lambda name: ctx.enter_context(nc.semaphore(name))

    # ---------------- SBUF (persistent) ----------------
    c_w1a = sb("c_w1a", [128, 2, NHID], BF16)
    c_w1b = sb("c_w1b", [128, 2, NHID], BF16)
    c_w2 = sb("c_w2", [128, 64], BF16)
    c_iota = sb("c_iota", [128, 128], BF16)
    c_idb = sb("c_idb", [128, 128], BF16)
    c_g1w = sb("c_g1w", [128, 128], BF16)
    c_g2w = sb("c_g2w", [128, 128], BF16)
    c_h1w = sb("c_h1w", [128, 64], F32)
    c_h2w = sb("c_h2w", [128, 64], F32)
    c_b1r = sb("c_b1r", [128, 128], BF16)
    c_b2r = sb("c_b2r", [128, 64], F32)
    cbias = sb("cbias", [128, 4], F32)

    gidx_sb = {a: sb(f"gidx{a}_sb", [128, nch[a] * 8], I16) for a in (1, 2)}
    dst_sb = {a: sb(f"dst{a}_sb", [128, nch[a]], F32) for a in (1, 2)}
    ew_sb = {a: sb(f"ew{a}_sb", [128, nch[a]], F32) for a in (1, 2)}

    msg = sb("msg", [128, RING * CALL_CHUNKS, 128], BF16)
    ohr = sb("ohr", [128, RING * CALL_CHUNKS, 128], BF16)
    stage = sb("stage", [128, 2, NB, 128], BF16)

    # phase A x buffers (freed before phase C region is allocated)
    sbA = ExitStack()
    xA = [sbA.enter_context(nc.sbuf_tensor(f"xA{i}", [128, 2, SP], BF16))
          for i in range(3)]

    psum = [ctx.enter_context(nc.psum_tensor(f"P{i}", [128, GROUP, 128], F32))
            for i in range(NPG)]

    io = Ctr(sem("io"), 16)        # SP-engine DMAs (one FIFO)
    gsems = [Ctr(sem(f"g{i}"), 16) for i in range(RING)]
    ccs = [Ctr(sem(f"cc{i}"), 1) for i in range(2)]
    pe = Ctr(sem("pe"), 1)
    dv = Ctr(sem("dv"), 1)
    ac = Ctr(sem("ac"), 1)
    gp = Ctr(sem("gp"), 1)

    SY, PE, DV, AC, GP = nc.sync, nc.tensor, nc.vector, nc.scalar, nc.gpsimd

    def fence():
        # SP waits for all its issued DMAs: later cross-engine io-threshold
        # waits become unambiguous (no completion reordering).
        SY.wait_ge(io.sem, io.n)

    # PSUM tile WAR tracking: each use of psum[i] appends its freeing ac val.
    psum_free = [None] * NPG       # ac val that frees this tile
    stage_io = [None, None]        # io val of last DMA draining stage slot

    # =========== Phase A: constants + local s tables ===========
    for bi, bval in enumerate((g1b, g2b, h1b, h2b)):
        DV.memset(cbias[:, bi:bi + 1], float(bval))
    for name, t in (("w1a", c_w1a), ("w1b", c_w1b), ("w2", c_w2),
                    ("iota", c_iota), ("idb", c_idb), ("g1w", c_g1w),
                    ("g2w", c_g2w), ("h1w", c_h1w), ("h2w", c_h2w),
                    ("b1r", c_b1r), ("b2r", c_b2r)):
        io.inc(SY.dma_start(t[:], din[name][:]))
    io.inc(SY.dma_start(xA[0][:], din["xt1a"][:]))
    io.inc(SY.dma_start(xA[1][:], din["xt1b"][:]))
    x1_io = io.n
    fence()
    for a in (1, 2):
        io.inc(SY.dma_start(gidx_sb[a][:], din[f"gidx{a}"][:]))
        io.inc(SY.dma_start(dst_sb[a][:], din[f"dst{a}"][:]))
        io.inc(SY.dma_start(ew_sb[a][:], din[f"eww{a}"][:]))
    idx_io = io.n
    fence()
    io.inc(SY.dma_start(xA[2][:], din["xt2a"][:]))
    x2a_io = io.n

    pg_user = [0]                  # global psum-group user counter

    def psum_acquire():
        """Returns psum tile for the next group user; emits PE wait if reused."""
        i = pg_user[0] % NPG
        pg_user[0] += 1
        return i, psum_free[i]

    tdma_io = {}

    def build_table(tbl, xa, xb, wa, wb, slot, pe_wait):
        """s = [xa@W1a | xb@W1b] for local shard -> stage[slot] -> DMA tdr."""
        acv0 = None
        for g7 in range(GROUP):
            pi, freev = psum_acquire()
            P = psum[pi]
            for b7 in range(GROUP):
                t = g7 * GROUP + b7
                first = (g7 == 0 and b7 == 0)
                if first and pe_wait is not None:
                    PE.wait_ge(io.sem, pe_wait)
                if b7 == 0 and freev is not None:
                    PE.wait_ge(ac.sem, freev)
                for xt, w, c0 in ((xa, wa, 0), (xb, wb, 64)):
                    for cch in range(2):
                        mm = PE.matmul(P[:, b7, c0:c0 + 64],
                                       xt[:, cch, t * 128:(t + 1) * 128],
                                       w[:, cch, :],
                                       start=(cch == 0), stop=(cch == 1))
            pe.inc(mm)
            AC.wait_ge(pe.sem, pe.n)
            if g7 == 0 and stage_io[slot] is not None:
                AC.wait_ge(io.sem, stage_io[slot])
            cp = AC.activation(
                stage[:, slot, g7 * GROUP:(g7 + 1) * GROUP, :], P[:],
                ACTF.Copy)
            ac.inc(cp)
            psum_free[pi] = ac.n
            if acv0 is None:
                acv0 = ac.n
        SY.wait_ge(ac.sem, ac.n)
        io.inc(SY.dma_start(
            tdr[tbl][:].rearrange("(t p) f -> p t f", p=128),
            stage[:, slot, :, :]))
        stage_io[slot] = io.n
        tdma_io[tbl] = io.n
        fence()

    build_table(1, xA[0], xA[1], c_w1a, c_w1b, 0, x1_io)
    t1_pe = pe.n
    # x2b reuses xA[0]: wait until t1 matmuls consumed it
    SY.wait_ge(pe.sem, t1_pe)
    io.inc(SY.dma_start(xA[0][:], din["xt2b"][:]))
    x2b_io = io.n
    fence()
    build_table(2, xA[2], xA[0], c_w1a, c_w1b, 1, max(x2a_io, x2b_io))

    # =========== edge pass machinery ===========
    gcall = [0]
    pe_cons = []

    def edge_pass(a, tbl, F, part, aslice, stg, stg_io):
        cpb = cpbs[a]
        nchunks = nch[a]
        # chunk -> (block, start, stop)
        sched = []
        for b in range(NBF):
            for i in range(int(cpb[b])):
                sched.append((b, i == 0, i == int(cpb[b]) - 1))
        assert len(sched) == nchunks
        gsb, dsb, esb = gidx_sb[a], dst_sb[a], ew_sb[a]
        tv = tdma_io[tbl]
        cur_psum = {}         # (r, g7) -> psum idx
        for j in range(nchunks // CALL_CHUNKS):
            slot = gcall[0] % RING
            q = gcall[0] % 2
            if j == 0:
                GP.wait_ge(io.sem, max(tv, idx_io))
            if len(pe_cons) >= RING:
                GP.wait_ge(pe.sem, pe_cons[-RING])
            g = GP.dma_gather(
                msg[:, slot * CALL_CHUNKS:(slot + 1) * CALL_CHUNKS, :], tdr[tbl][:],
                gsb[:, j * (S_CALL // 16):(j + 1) * (S_CALL // 16)],
                S_CALL, S_CALL, 128, queue_num=q)
            gsems[slot].inc(g)
            gv = gsems[slot].n
            if j == 0:
                DV.wait_ge(io.sem, idx_io)
            if len(pe_cons) >= RING:
                DV.wait_ge(pe.sem, pe_cons[-RING])
            for c8 in range(CALL_CHUNKS):
                c = j * CALL_CHUNKS + c8
                ts = DV.tensor_scalar(
                    ohr[:, slot * CALL_CHUNKS + c8, :], c_iota[:],
                    dsb[:, c:c + 1], esb[:, c:c + 1],
                    op0=AOP.is_equal, op1=AOP.mult)
            dv.inc(ts)
            ohv = dv.n
            PE.wait_ge(gsems[slot].sem, gv)
            PE.wait_ge(dv.sem, ohv)
            last_inc = False
            for c8 in range(CALL_CHUNKS):
                c = j * CALL_CHUNKS + c8
                b, st, sp = sched[c]
                r, bl = b // NB, b % NB
                g7, b7 = bl // GROUP, bl % GROUP
                if st and b7 == 0:
                    pi, freev = psum_acquire()
                    cur_psum[(r, g7)] = pi
                    if freev is not None:
                        PE.wait_ge(ac.sem, freev)
                pi = cur_psum[(r, g7)]
                P = psum[pi]
                mm = PE.matmul(P[:, b7, 0:F],
                               ohr[:, slot * CALL_CHUNKS + c8, :],
                               msg[:, slot * CALL_CHUNKS + c8, 0:F],
                               start=st, stop=sp)
                last_inc = False
                if sp and b7 == GROUP - 1:
                    pe.inc(mm)
                    last_inc = True
                    # emit group copy now (ACT in-order)
                    AC.wait_ge(pe.sem, pe.n)
                    if g7 == 0 and stg_io[r % 2] is not None:
                        AC.wait_ge(io.sem, stg_io[r % 2])
                    cp = AC.activation(
                        stg[:, r % 2, g7 * GROUP:(g7 + 1) * GROUP, :],
                        P[:, :, 0:F], ACTF.Copy)
                    ac.inc(cp)
                    psum_free[pi] = ac.n
                    if g7 == GROUP - 1:
                        SY.wait_ge(ac.sem, ac.n)
                        io.inc(SY.dma_start(
                            part[(2 * r + aslice) * SP:(2 * r + aslice + 1) * SP, 0:F]
                            .rearrange("(t p) f -> p t f", p=128),
                            stg[:, r % 2, :, :]))
                        stg_io[r % 2] = io.n
                        fence()
            if not last_inc:
                pe.inc(mm)
            pe_cons.append(pe.n)
            gcall[0] += 1
        return io.n

    edge_pass(1, 1, 128, part1, 0, stage, stage_io)
    p2io = edge_pass(2, 2, 128, part1, 1, stage, stage_io)

    # =========== ReduceScatter 1 (both adjacencies, layer 1) ===========
    GP.wait_ge(io.sem, p2io)
    ccs[0].inc(GP.collective_compute(
        "ReduceScatter", AOP.add, replica_groups=[list(range(NCORES))],
        ins=[part1[:]], outs=[o12[:]]))

    # =========== Phase C: gating + L2 table (all local) ===========
    sbA.close()
    sbC = ExitStack()
    sc = lambda name, shape, dt: sbC.enter_context(nc.sbuf_tensor(name, shape, dt))
    x1c = sc("x1c", [128, NB, 128], BF16)
    x2c = sc("x2c", [128, NB, 128], BF16)
    tmpb = sc("tmpb", [128, NB, 128], BF16)
    xtt = sc("xtt", [128, 2, 128], BF16)
    lam = {nm: sc(f"lam_{nm}", [128, NB], F32) for nm in ("l1", "l2", "ls", "w0", "w1")}
    w0h = sc("w0h", [128, NB], BF16)
    w1h = sc("w1h", [128, NB], BF16)
    stage2 = sc("stage2", [128, 2, NB, 64], BF16)
    p1b = sc("p1b", [128, NB, NCLASS], BF16)
    p2b = sc("p2b", [128, NB, NCLASS], BF16)
    p1f = sc("p1f", [128, NB, NCLASS], F32)
    p2f = sc("p2f", [128, NB, NCLASS], F32)
    t32 = sc("t32", [128, NB, NCLASS], F32)

    SY.wait_ge(ccs[0].sem, 1)
    io.inc(SY.dma_start(x1c[:], o12[0:SP].rearrange("(t p) f -> p t f", p=128)))
    io.inc(SY.dma_start(x2c[:], o12[SP:2 * SP].rearrange("(t p) f -> p t f", p=128)))
    xc_io = io.n
    fence()

    b1b = c_b1r[:, None, :].broadcast_to([128, NB, 128])
    # x1 on DVE, x2 on GPSIMD concurrently
    DV.wait_ge(io.sem, xc_io)
    DV.tensor_tensor(x1c[:], x1c[:], b1b, op=AOP.add)
    DV.drain()
    DV.tensor_scalar(x1c[:], x1c[:], 0.0, None, op0=AOP.max)
    DV.drain()
    GP.wait_ge(io.sem, xc_io)
    GP.tensor_tensor(x2c[:], x2c[:], b1b, op=AOP.add)
    GP.drain()
    gp.inc(GP.tensor_scalar(x2c[:], x2c[:], 0.0, None, op0=AOP.max))
    g1b_b = c_g1w[:, None, :].broadcast_to([128, NB, 128])
    g2b_b = c_g2w[:, None, :].broadcast_to([128, NB, 128])
    DV.tensor_tensor(tmpb[:], x1c[:], g1b_b, op=AOP.mult)
    DV.drain()
    DV.tensor_reduce(lam["l1"][:], tmpb[:], axis=mybir.AxisListType.X, op=AOP.add)
    DV.drain()
    DV.wait_ge(gp.sem, gp.n)
    DV.tensor_tensor(tmpb[:], x2c[:], g2b_b, op=AOP.mult)
    DV.drain()
    dv.inc(DV.tensor_reduce(lam["l2"][:], tmpb[:], axis=mybir.AxisListType.X,
                            op=AOP.add))
    AC.wait_ge(dv.sem, dv.n)
    AC.activation(lam["l1"][:], lam["l1"][:], ACTF.Sigmoid, bias=cbias[:, 0:1])
    ac.inc(AC.activation(lam["l2"][:], lam["l2"][:], ACTF.Sigmoid, bias=cbias[:, 1:2]))
    DV.wait_ge(ac.sem, ac.n)
    DV.tensor_tensor(lam["ls"][:], lam["l1"][:], lam["l2"][:], op=AOP.add)
    DV.drain()
    DV.tensor_scalar(lam["ls"][:], lam["ls"][:], 1e-12, None, op0=AOP.max)
    DV.drain()
    DV.reciprocal(lam["ls"][:], lam["ls"][:])
    DV.drain()
    DV.tensor_tensor(lam["w0"][:], lam["l1"][:], lam["ls"][:], op=AOP.mult)
    DV.tensor_tensor(lam["w1"][:], lam["l2"][:], lam["ls"][:], op=AOP.mult)
    DV.drain()
    DV.tensor_copy(w0h[:], lam["w0"][:])
    DV.tensor_copy(w1h[:], lam["w1"][:])
    DV.drain()
    w0b = w0h[:, :, None].broadcast_to([128, NB, 128])
    w1b = w1h[:, :, None].broadcast_to([128, NB, 128])
    DV.tensor_tensor(x1c[:], x1c[:], w0b, op=AOP.mult)
    DV.tensor_tensor(tmpb[:], x2c[:], w1b, op=AOP.mult)
    DV.drain()
    dv.inc(DV.tensor_tensor(x1c[:], x1c[:], tmpb[:], op=AOP.add))
    xfin = dv.n
    # zero right half of t3 staging (slot 0)
    DV.wait_ge(io.sem, stage_io[0])
    dv.inc(DV.memset(stage[:, 0, :, 64:128], 0))
    zfin = dv.n

    # L2 table: s2 = x @ W2 via per-block transpose pipeline
    dvv, mmv, tac = {}, {}, {}
    for t in range(NB):
        pi = [0, 1][t % 2]
        pj = [2, 3][t % 2]
        if t < 2:
            PE.wait_ge(dv.sem, xfin)
            if psum_free[pi] is not None:
                PE.wait_ge(ac.sem, psum_free[pi])
            if psum_free[pj] is not None:
                PE.wait_ge(ac.sem, psum_free[pj])
        else:
            PE.wait_ge(dv.sem, dvv[t - 2])      # psum[pi] reuse
        pbf = psum[pi][:, 0, 0:64].bitcast(BF16)
        pe.inc(PE.transpose(pbf, x1c[:, t, :], c_idb[:]))
        DV.wait_ge(pe.sem, pe.n)
        if t >= 2:
            DV.wait_ge(pe.sem, mmv[t - 2])      # xtt slot reuse
        dv.inc(DV.tensor_copy(xtt[:, t % 2, :], pbf))
        dvv[t] = dv.n
        PE.wait_ge(dv.sem, dv.n)
        if t >= 2:
            PE.wait_ge(ac.sem, tac[t - 2])      # psum[pj] reuse
        pe.inc(PE.matmul(psum[pj][:, 0, 0:64], xtt[:, t % 2, :], c_w2[:],
                         start=True, stop=True))
        mmv[t] = pe.n
        AC.wait_ge(pe.sem, pe.n)
        if t == 0:
            AC.wait_ge(dv.sem, zfin)
        ac.inc(AC.activation(stage[:, 0, t, 0:64], psum[pj][:, 0, 0:64],
                             ACTF.Copy))
        tac[t] = ac.n
    for i in range(NPG):
        psum_free[i] = ac.n
    SY.wait_ge(ac.sem, ac.n)
    io.inc(SY.dma_start(
        tdr[3][:].rearrange("(t p) f -> p t f", p=128), stage[:, 0, :, :]))
    stage_io[0] = io.n
    tdma_io[3] = io.n
    fence()

    # =========== L2 edge passes ===========
    stage2_io = [None, None]
    edge_pass(1, 3, 64, part2, 0, stage2, stage2_io)
    p4io = edge_pass(2, 3, 64, part2, 1, stage2, stage2_io)

    # =========== ReduceScatter 2 (both adjacencies, layer 2) ===========
    GP.wait_ge(io.sem, p4io)
    ccs[1].inc(GP.collective_compute(
        "ReduceScatter", AOP.add, replica_groups=[list(range(NCORES))],
        ins=[part2[:]], outs=[o34[:]]))

    # =========== Final gating + outputs ===========
    SY.wait_ge(ccs[1].sem, 1)
    io.inc(SY.dma_start(p1b[:], o34[0:SP, 0:NCLASS]
                        .rearrange("(t p) f -> p t f", p=128)))
    io.inc(SY.dma_start(p2b[:], o34[SP:2 * SP, 0:NCLASS]
                        .rearrange("(t p) f -> p t f", p=128)))
    pb_io = io.n
    fence()
    b2b = c_b2r[:, None, 0:NCLASS].broadcast_to([128, NB, NCLASS])
    DV.wait_ge(io.sem, pb_io)
    DV.tensor_tensor(p1f[:], p1b[:], b2b, op=AOP.add)
    dv.inc(DV.tensor_tensor(p2f[:], p2b[:], b2b, op=AOP.add))
    pfin = dv.n
    DV.drain()
    SY.wait_ge(dv.sem, pfin)
    io.inc(SY.dma_start(p1_o[:].rearrange("(t p) f -> p t f", p=128), p1f[:]))
    io.inc(SY.dma_start(p2_o[:].rearrange("(t p) f -> p t f", p=128), p2f[:]))
    pout_io = io.n
    fence()
    h1b_b = c_h1w[:, None, 0:NCLASS].broadcast_to([128, NB, NCLASS])
    h2b_b = c_h2w[:, None, 0:NCLASS].broadcast_to([128, NB, NCLASS])
    DV.tensor_tensor(t32[:], p1f[:], h1b_b, op=AOP.mult)
    DV.drain()
    DV.tensor_reduce(lam["l1"][:], t32[:], axis=mybir.AxisListType.X, op=AOP.add)
    DV.drain()
    DV.tensor_tensor(t32[:], p2f[:], h2b_b, op=AOP.mult)
    DV.drain()
    dv.inc(DV.tensor_reduce(lam["l2"][:], t32[:], axis=mybir.AxisListType.X,
                            op=AOP.add))
    AC.wait_ge(dv.sem, dv.n)
    AC.activation(lam["l1"][:], lam["l1"][:], ACTF.Sigmoid, bias=cbias[:, 2:3])
    ac.inc(AC.activation(lam["l2"][:], lam["l2"][:], ACTF.Sigmoid, bias=cbias[:, 3:4]))
    DV.wait_ge(ac.sem, ac.n)
    DV.tensor_tensor(lam["ls"][:], lam["l1"][:], lam["l2"][:], op=AOP.add)
    DV.drain()
    DV.tensor_scalar(lam["ls"][:], lam["ls"][:], 1e-12, None, op0=AOP.max)
    DV.drain()
    DV.reciprocal(lam["ls"][:], lam["ls"][:])
    DV.drain()
    DV.tensor_tensor(lam["w0"][:], lam["l1"][:], lam["ls"][:], op=AOP.mult)
    DV.tensor_tensor(lam["w1"][:], lam["l2"][:], lam["ls"][:], op=AOP.mult)
    DV.drain()
    w0f = lam["w0"][:, :, None].broadcast_to([128, NB, NCLASS])
    w1f = lam["w1"][:, :, None].broadcast_to([128, NB, NCLASS])
    DV.wait_ge(io.sem, pout_io)
    DV.tensor_tensor(p1f[:], p1f[:], w0f, op=AOP.mult)
    DV.tensor_tensor(p2f[:], p2f[:], w1f, op=AOP.mult)
    DV.drain()
    dv.inc(DV.tensor_tensor(p1f[:], p1f[:], p2f[:], op=AOP.add))
    SY.wait_ge(dv.sem, dv.n)
    io.inc(SY.dma_start(out_o[:].rearrange("(t p) f -> p t f", p=128), p1f[:]))
    SY.wait_ge(io.sem, io.n)

    nc.compile()
    sbC.close()
    ctx.close()
    return nc


def _prep_all(inputs):
    adj = {}
    cpbs = {}
    for a in (1, 2):
        out, cpb = _prep_adjacency(inputs[f"src{a}"], inputs[f"dst{a}"],
                                   inputs[f"ew{a}"])
        adj[a] = out
        cpbs[a] = cpb
    return adj, cpbs


def _in_maps(inputs, adj):
    bf = ml_dtypes.bfloat16
    f32 = np.float32

    def wfmt(w):  # [256, 64] -> [128, 2, 64] bf16
        return np.ascontiguousarray(
            np.asarray(w, f32).reshape(2, 128, NHID).transpose(1, 0, 2)).astype(bf)

    w2pad = np.zeros((128, 64), f32)
    w2pad[:, :NCLASS] = np.asarray(inputs["W2"], f32)
    iota = np.tile(np.arange(128, dtype=f32), (128, 1))
    ident = np.eye(128, dtype=f32)
    g1w = np.tile(np.asarray(inputs["g1w"], f32).ravel(), (128, 1))
    g2w = np.tile(np.asarray(inputs["g2w"], f32).ravel(), (128, 1))
    h1w = np.zeros((128, 64), f32)
    h1w[:, :NCLASS] = np.asarray(inputs["h1w"], f32).ravel()
    h2w = np.zeros((128, 64), f32)
    h2w[:, :NCLASS] = np.asarray(inputs["h2w"], f32).ravel()
    b1r = np.tile(np.concatenate([np.asarray(inputs["b1a"], f32).ravel(),
                                  np.asarray(inputs["b1b"], f32).ravel()]), (128, 1))
    b2r = np.zeros((128, 64), f32)
    b2r[:, :NCLASS] = np.asarray(inputs["b2"], f32).ravel()

    common = dict(
        w1a=wfmt(inputs["W1a"]), w1b=wfmt(inputs["W1b"]),
        w2=w2pad.astype(bf), iota=iota.astype(bf), idb=ident.astype(bf),
        g1w=g1w.astype(bf), g2w=g2w.astype(bf), h1w=h1w, h2w=h2w,
        b1r=b1r.astype(bf), b2r=b2r)

    def xfmt(x, k):  # shard k, pad, transpose -> [128, 2, SP] bf16
        xs = np.asarray(x, f32)[k * S:(k + 1) * S]
        xp = np.zeros((SP, NFEAT), f32)
        xp[:S] = xs
        xt = xp.T.reshape(2, 128, SP).transpose(1, 0, 2)
        return np.ascontiguousarray(xt).astype(bf)

    in_maps = []
    for k in range(NCORES):
        m = dict(common)
        for v, key in (("xt1a", "x1a"), ("xt1b", "x1b"),
                       ("xt2a", "x2a"), ("xt2b", "x2b")):
            m[v] = xfmt(inputs[key], k)
        for a in (1, 2):
            g, d, e = adj[a][k]
            m[f"gidx{a}"] = g
            m[f"dst{a}"] = d
            m[f"eww{a}"] = e
        in_maps.append(m)
    return in_maps


def _run(inputs, sim=False):
    adj, cpbs = _prep_all(inputs)
    scalars = (float(np.asarray(inputs["g1b"]).ravel()[0]),
               float(np.asarray(inputs["g2b"]).ravel()[0]),
               float(np.asarray(inputs["h1b"]).ravel()[0]),
               float(np.asarray(inputs["h2b"]).ravel()[0]))
    nc = _build(cpbs, scalars)
    in_maps = _in_maps(inputs, adj)

    global LAST_EXEC_NS
    if sim:
        from concourse.bass_interp import MultiCoreSim
        msim = MultiCoreSim(nc, NCORES)
        for k in range(NCORES):
            for name, arr in in_maps[k].items():
                msim.cores[k].tensor(name)[:] = arr
        msim.simulate()
        results = [{nm: msim.cores[k].tensor(nm).copy()
                    for nm in ("out_o", "p1_o", "p2_o")} for k in range(NCORES)]
    else:
        r = run_bass_kernel_spmd(nc, in_maps, list(range(NCORES)))
        LAST_EXEC_NS = r.exec_time_ns
        results = r.results

    outs = []
    for nm in ("out_o", "p1_o", "p2_o"):
        outs.append(np.concatenate([results[k][nm][:S] for k in range(NCORES)],
                    axis=0).astype(np.float32))
    return tuple(outs)


LAST_EXEC_NS = None
LAST_WALL2_S = None


def kernel(**inputs):
    return _run(inputs, sim=False)
